# revision 11
# baseline (speedup 1.0000x reference)
"""Trainium2 Bass kernel: multi-head self-attention with RoPE, causal mask.

Reference semantics (B=2, S=2048, D=1024, H=16, DK=64):
    q = rope(x @ Wq.T), k = rope(x @ Wk.T), v = x @ Wv.T   (per-head views)
    out = softmax(causal(q k^T / 8)) v ;  y = out @ Wo.T

Sharding over 8 cores: 2-way batch x 4-way heads (4 heads/core).
Each core computes a partial y [S, D] (its heads' contribution); host sums
the 4 partials per batch (device output is fp16, summed in fp64 on host).

On-device layout strategy (per core):
  - host passes xT = x[b].T [1024, 2048]; ALL 16-bit operands are fp16
    (better mantissa than bf16, same 1-cycle/row PE rate, 2x DVE rate)
  - Q/K projected ONCE; the rotate-half partner comes from a per-512-chunk
    SBUF-to-SBUF DMA block swap; rope = qa*cos (DVE) + qas*sin (gpsimd),
    add on DVE; PSUM evacuations ride the ACT engine
  - attention is processed HEAD-PAIR-OUTER: all of head-pair 0's q-groups
    run right after V + K(ec0) + Q(ec0) finish, with the K(ec1)/Q(ec1)
    projection chunks interleaved INTO that stream as PE filler (they use
    the out-projection's idle PSUM slot); head-pair 1 then runs with the
    output projection interleaved as filler
  - scores are computed TRANSPOSED (k on partitions, q on free); both
    heads of a pair write into ONE two-bank PSUM tile so a single Exp
    activation covers both (ACT instruction count halved; ACT binds)
  - V tiles are 128-wide blocks: ones column at col 0 (softmax denominator
    lands on PSUM partition 0 where the fast approx reciprocal works), V
    data at cols 64..127 (partition-base-64 aligned for the DVE multiply)
  - normalization: reciprocal_approx_fast + f32r copy + PE broadcast
    matmul, multiply deferred into the next stream's slack
"""

import sys

sys.path.insert(0, "/opt/trn_rl_repo")

import numpy as np
import ml_dtypes


S = 2048
D = 1024
NH = 16
DK = 64
HL = 4          # heads per core
EL = HL * DK    # 256 local e-dims
N_CORES = 8
THETA = 10000.0

_compiled = None


def _round_fp32r(x):
    # fp32r matmul operands must be pre-rounded to 11 mantissa bits (RNE)
    xi = np.ascontiguousarray(x, dtype=np.float32).view(np.uint32).astype(np.uint64)
    bias = ((xi >> 12) & 1) + (1 << 11) - 1
    return ((xi + bias) >> 12 << 12).astype(np.uint32).view(np.float32)


def _build():
    import concourse.bacc as bacc
    import concourse.tile as tile
    from concourse import mybir
    from concourse.alu_op_type import AluOpType

    dt = mybir.dt
    f32, f32r = dt.float32, dt.float32r
    f16 = dt.float16

    nc = bacc.Bacc("TRN2", target_bir_lowering=False, debug=False,
                   num_devices=N_CORES)

    xT = nc.dram_tensor("xT", [D, S], f16, kind="ExternalInput").ap()
    wq = nc.dram_tensor("wq", [D, EL], f16, kind="ExternalInput").ap()
    wk = nc.dram_tensor("wk", [D, EL], f16, kind="ExternalInput").ap()
    wv = nc.dram_tensor("wv", [D, EL], f16, kind="ExternalInput").ap()
    wo = nc.dram_tensor("wo", [EL, D], f16, kind="ExternalInput").ap()
    cosT = nc.dram_tensor("cosT", [128, S], f16, kind="ExternalInput").ap()
    sinT = nc.dram_tensor("sinT", [128, S], f16, kind="ExternalInput").ap()
    ident = nc.dram_tensor("ident", [128, 128], f32r, kind="ExternalInput").ap()
    y = nc.dram_tensor("y", [S, D], f16, kind="ExternalOutput").ap()

    with tile.TileContext(nc) as tc:
        with tc.tile_pool(name="persist", bufs=1) as pp:
            # persistent SBUF tiles (live across both stages)
            qt = [pp.tile([128, S], f16, tag=f"qt{c}", name=f"qt{c}") for c in range(2)]
            ktz = [[pp.tile([128, S], f16, tag=f"ktz{c}{par}", name=f"ktz{c}{par}")
                   for par in range(2)] for c in range(2)]
            vh = [pp.tile([128, 16 * 128], f16, tag=f"v{h}", name=f"v{h}") for h in range(HL)]
            cos_sb = pp.tile([128, S], f16, tag="cos", name="cos")
            sin_sb = pp.tile([128, S], f16, tag="sin", name="sin")
            id_sb = pp.tile([128, 128], f32r, tag="ident", name="ident")
            xts = [pp.tile([128, S], f16, tag=f"xt{dc}", name=f"xt{dc}") for dc in range(8)]
            wks = [pp.tile([128, EL], f16, tag=f"wk{dc}", name=f"wk{dc}") for dc in range(8)]
            wqs = [pp.tile([128, EL], f16, tag=f"wq{dc}", name=f"wq{dc}") for dc in range(8)]

            def load_xt_slices(sg, split=1):
                for dc in range(8):
                    eng = nc.sync if dc % 2 == 0 else nc.scalar
                    p = 128 // split
                    for j in range(split):
                        eng.dma_start(
                            xts[dc][p * j:p * (j + 1), 512 * sg:512 * (sg + 1)],
                            xT[128 * dc + p * j:128 * dc + p * (j + 1),
                               512 * sg:512 * (sg + 1)])

            # rope chunk: evacuate PSUM proj (ACT), DMA block-swap to build
            # the rotate-half partner, cos/sin multiplies (DVE + gpsimd),
            # add into the destination K/Z or Q tile (DVE)
            def rope_chunk(ps, qa, qas, sg, is_k, ec, ropool):
                sl = slice(512 * sg, 512 * (sg + 1))
                nc.scalar.copy(qa[:, sl], ps[:])
                for blk in range(2):
                    b0 = 64 * blk
                    nc.sync.dma_start(
                        qas[b0:b0 + 32, sl], qa[b0 + 32:b0 + 64, sl])
                    nc.sync.dma_start(
                        qas[b0 + 32:b0 + 64, sl], qa[b0:b0 + 32, sl])
                qc = ropool.tile([128, 512], f16, tag="qc", name="qc")
                qs = ropool.tile([128, 512], f16, tag="qs", name="qs")
                nc.vector.tensor_mul(qc[:], qa[:, sl], cos_sb[:, sl])
                nc.gpsimd.tensor_mul(qs[:], qas[:, sl], sin_sb[:, sl])
                if is_k:
                    # zero-padded K halves so score matmuls see K=128 rows
                    nc.vector.tensor_add(
                        ktz[ec][0][0:64, sl], qc[0:64, :], qs[0:64, :])
                    nc.vector.tensor_add(
                        ktz[ec][1][64:128, sl], qc[64:128, :], qs[64:128, :])
                else:
                    nc.vector.tensor_add(qt[ec][:, sl], qc[:], qs[:])

            # ======== stage A: V (all) + K/Q ec0 projections ========
            with tc.tile_pool(name="stagea", bufs=1) as sa, \
                 tc.tile_pool(name="qap", bufs=2) as qap, \
                 tc.tile_pool(name="ropa", bufs=3) as ropa, \
                 tc.tile_pool(name="vtp", bufs=2) as vtp, \
                 tc.tile_pool(name="pa", bufs=4, space="PSUM") as pap, \
                 tc.tile_pool(name="tr", bufs=2, space="PSUM") as trp:

                # warm up the PE clock-gate while input DMAs land
                nc.scalar.dma_start(id_sb[:], ident)
                idb = id_sb[:].bitcast(dt.bfloat16)   # values irrelevant
                wp = trp.tile([128, 512], f32, tag="warm", name="warm")
                for _ in range(50):
                    nc.tensor.matmul(wp[:, 0:256], idb[:, 0:128], idb[:],
                                     start=True, stop=True)

                load_xt_slices(0, split=2)
                wvs = [sa.tile([128, EL], f16, tag=f"w{dc}", name=f"w{dc}")
                       for dc in range(8)]
                for dc in range(8):
                    nc.sync.dma_start(wvs[dc][:], wv[128 * dc:128 * (dc + 1), :])
                load_xt_slices(1)
                load_xt_slices(2)
                load_xt_slices(3)
                for dc in range(8):
                    nc.sync.dma_start(wks[dc][:], wk[128 * dc:128 * (dc + 1), :])
                for half in range(2):
                    sl = slice(1024 * half, 1024 * (half + 1))
                    nc.scalar.dma_start(cos_sb[:, sl], cosT[:, sl])
                    nc.scalar.dma_start(sin_sb[:, sl], sinT[:, sl])
                for dc in range(8):
                    nc.scalar.dma_start(wqs[dc][:], wq[128 * dc:128 * (dc + 1), :])

                ones16 = sa.tile([128, 16], f32, tag="ones16", name="ones16")
                nc.vector.memset(ones16[:], 1.0)
                for c in range(2):
                    nc.gpsimd.memset(ktz[c][0][64:128, :], 0.0)
                    nc.gpsimd.memset(ktz[c][1][0:64, :], 0.0)

                # --- V projection (VT then PE-transpose into [s, dv]) ---
                for sg in range(4):
                    for ec in range(2):
                        ps = pap.tile([128, 512], f32, tag="pa", name="pa")
                        for dc in range(8):
                            nc.tensor.matmul(
                                ps[:],
                                wvs[dc][:, 128 * ec:128 * (ec + 1)],
                                xts[dc][:, 512 * sg:512 * (sg + 1)],
                                start=(dc == 0), stop=(dc == 7))
                        vt = vtp.tile([128, 512], f32r, tag="vt", name="vt")
                        nc.vector.tensor_copy(vt[:], ps[:])
                        tr = trp.tile([128, 512], f32r, tag="tr", name="tr")
                        for i in range(4):
                            nc.tensor.transpose(
                                tr[:, 128 * i:128 * (i + 1)],
                                vt[:, 128 * i:128 * (i + 1)], id_sb[:])
                        for i in range(4):
                            sc = 4 * sg + i
                            for hh in range(2):
                                h = 2 * ec + hh
                                nc.scalar.copy(
                                    vh[h][:, 128 * sc + 64:128 * sc + 128],
                                    tr[:, 128 * i + 64 * hh:128 * i + 64 * hh + 64])
                for h in range(HL):
                    ones_col = vh[h][:].rearrange(
                        "p (s c) -> p s c", c=128)[:, :, 0:1]
                    nc.vector.tensor_copy(ones_col, ones16[:].rearrange("p (s c) -> p s c", c=1))

                # --- K ec0 then Q ec0 (head pair 0's tensors) ---
                for is_k, ws in [(True, wks), (False, wqs)]:
                    qa = qap.tile([128, S], f16, tag="qa", name="qa")
                    qas = qap.tile([128, S], f16, tag="qas", name="qas")
                    for sg in range(4):
                        ps = pap.tile([128, 512], f32, tag="pa", name="pa")
                        for dc in range(8):
                            nc.tensor.matmul(
                                ps[:],
                                ws[dc][:, 0:128],
                                xts[dc][:, 512 * sg:512 * (sg + 1)],
                                start=(dc == 0), stop=(dc == 7))
                        rope_chunk(ps, qa, qas, sg, is_k, 0, ropa)

            # ======== stage B: attention (head-pair outer) + out-proj ========
            with tc.tile_pool(name="pb", bufs=1) as pb, \
                 tc.tile_pool(name="ptp", bufs=8) as ptp, \
                 tc.tile_pool(name="nrm", bufs=4) as nrmp, \
                 tc.tile_pool(name="ysb", bufs=2) as ysbp, \
                 tc.tile_pool(name="kqp", bufs=2) as kqp, \
                 tc.tile_pool(name="ropb", bufs=3) as ropb, \
                 tc.tile_pool(name="ps_s", bufs=2, space="PSUM") as ps_s, \
                 tc.tile_pool(name="ps_pv", bufs=2, space="PSUM") as ps_pv, \
                 tc.tile_pool(name="ps_y", bufs=1, space="PSUM") as ps_y, \
                 tc.tile_pool(name="ps_bc", bufs=1, space="PSUM") as ps_bc:

                aot = [pb.tile([128, S], f16, tag=f"aot{c}", name=f"aot{c}") for c in range(2)]
                wo_sb = [pb.tile([128, D], f16, tag=f"wo{c}", name=f"wo{c}") for c in range(2)]
                ones_sb = pb.tile([1, 64], f32r, tag="ones", name="ones")
                onesf = pb.tile([1, 64], f32, tag="onesf", name="onesf")
                nc.vector.memset(onesf[:], 1.0)
                nc.vector.tensor_copy(ones_sb[:], onesf[:])
                for c in range(2):
                    for half in range(2):
                        sl = slice(512 * half, 512 * (half + 1))
                        nc.scalar.dma_start(wo_sb[c][:, sl],
                                            wo[128 * c:128 * (c + 1), sl])

                # ec1 projection chunks, emitted as PE filler inside head-
                # pair 0's attention stream (use the out-projection's PSUM
                # slot, which is idle until head-pair 1)
                kq_tiles = {}
                for is_k in (True, False):
                    kq_tiles[is_k] = (
                        kqp.tile([128, S], f16, tag="qa", name="qa"),
                        kqp.tile([128, S], f16, tag="qas", name="qas"))

                def mk_proj_heat(is_k, sg):
                    def emit():
                        ws = wks if is_k else wqs
                        qa, qas = kq_tiles[is_k]
                        ps = ps_y.tile([128, 512], f32, tag="yp", name="yp")
                        for dc in range(8):
                            nc.tensor.matmul(
                                ps[:],
                                ws[dc][:, 128:256],
                                xts[dc][:, 512 * sg:512 * (sg + 1)],
                                start=(dc == 0), stop=(dc == 7))
                        rope_chunk(ps, qa, qas, sg, is_k, 1, ropb)
                    return emit

                proj_heats = [mk_proj_heat(True, sg) for sg in range(4)] + \
                             [mk_proj_heat(False, sg) for sg in range(4)]

                def out_proj_chunk(sc, split=False):
                    ysb = ysbp.tile([128, D], f16, tag="ysb", name="ysb")
                    for eg in range(2):
                        yp = ps_y.tile([128, 512], f32, tag="yp", name="yp")
                        for c2 in range(2):
                            nc.tensor.matmul(
                                yp[:],
                                aot[c2][:, 128 * sc:128 * (sc + 1)],
                                wo_sb[c2][:, 512 * eg:512 * (eg + 1)],
                                start=(c2 == 0), stop=(c2 == 1))
                        nc.vector.tensor_copy(
                            ysb[:, 512 * eg:512 * (eg + 1)], yp[:])
                    for half in range(2):
                        sl = slice(512 * half, 512 * (half + 1))
                        if split:
                            nc.sync.dma_start(
                                y[128 * sc:128 * sc + 64, sl], ysb[0:64, sl])
                            nc.scalar.dma_start(
                                y[128 * sc + 64:128 * (sc + 1), sl], ysb[64:128, sl])
                        else:
                            nc.sync.dma_start(
                                y[128 * sc:128 * (sc + 1), sl], ysb[:, sl])

                pending = []   # deferred normalize closures

                def emit_pending_one():
                    if pending:
                        pending.pop(0)()

                # per (hp, qg) slot schedules: which filler runs at which kc
                heat_sched = {
                    (0, 0): {1: proj_heats[0], 3: proj_heats[1]},
                    (0, 1): {3: proj_heats[2], 6: proj_heats[3]},
                    (0, 2): {4: proj_heats[4], 8: proj_heats[5]},
                    (0, 3): {5: proj_heats[6], 10: proj_heats[7]},
                    (1, 1): {2: lambda: out_proj_chunk(0), 4: lambda: out_proj_chunk(1),
                             5: lambda: out_proj_chunk(2), 6: lambda: out_proj_chunk(3)},
                    (1, 2): {2: lambda: out_proj_chunk(4), 5: lambda: out_proj_chunk(5),
                             8: lambda: out_proj_chunk(6), 10: lambda: out_proj_chunk(7)},
                    (1, 3): {2: lambda: out_proj_chunk(8), 6: lambda: out_proj_chunk(9),
                             10: lambda: out_proj_chunk(10), 13: lambda: out_proj_chunk(11)},
                }

                SKEW = 3
                for hp in range(2):
                    for qg in range(4):
                        n_kc = 4 * qg + 4
                        heats = heat_sched.get((hp, qg), {})
                        # flush the previous stream's two normalizes early
                        norm_at = {} if (hp, qg) == (0, 0) else {1: 2}
                        ppv = {}
                        for hh in range(2):
                            h = 2 * hp + hh
                            ppv[h] = ps_pv.tile([128, 512], f32, tag="ppv", name="ppv")
                        ptq = {}
                        for kc in range(n_kc + SKEW):
                            for _ in range(norm_at.get(kc, 0)):
                                emit_pending_one()
                            if kc in heats:
                                heats[kc]()
                            # PV first: keeps queued work ahead of a score
                            # matmul that may block on PSUM reuse
                            kcp = kc - SKEW
                            if kcp >= 0:
                                ptv2, q0v = ptq.pop(kcp)
                                for hh in range(2):
                                    h = 2 * hp + hh
                                    nc.tensor.matmul(
                                        ppv[h][:, q0v:512],
                                        vh[h][:, 128 * kcp:128 * kcp + 128],
                                        ptv2[:, 512 * hh + q0v:512 * (hh + 1)],
                                        start=(kcp == 0), stop=(kcp == n_kc - 1))
                            if kc < n_kc:
                                # diagonal tiles only need q >= k
                                r = kc - 4 * qg
                                q0 = 128 * r if r > 0 else 0
                                qsl = slice(512 * qg + q0, 512 * (qg + 1))
                                ps2 = ps_s.tile([128, 1024], f32, tag="ps", name="ps")
                                for hh in range(2):
                                    nc.tensor.matmul(
                                        ps2[:, 512 * hh + q0:512 * (hh + 1)],
                                        ktz[hp][hh][:, 128 * kc:128 * (kc + 1)],
                                        qt[hp][:, qsl],
                                        start=True, stop=True)
                                pt = ptp.tile([128, 1024], f16, tag="pt", name="pt")
                                psv = ps2[:].rearrange("p (h q) -> p h q", h=2)[:, :, q0:512]
                                ptv = pt[:].rearrange("p (h q) -> p h q", h=2)[:, :, q0:512]
                                nc.scalar.activation(
                                    ptv, psv,
                                    mybir.ActivationFunctionType.Exp,
                                    scale=0.125)
                                if r >= 0:
                                    for hh in range(2):
                                        nc.gpsimd.affine_select(
                                            pt[:, 512 * hh + q0:512 * (hh + 1)],
                                            pt[:, 512 * hh + q0:512 * (hh + 1)],
                                            pattern=[[1, 512 - q0]],
                                            compare_op=AluOpType.is_ge, fill=0.0,
                                            base=512 * qg + q0 - 128 * kc,
                                            channel_multiplier=-1)
                                ptq[kc] = (pt, q0)
                        # evacuate ppv fast: BOTH attn-out+denom copies first
                        # (they gate PSUM reuse), then the cheap reciprocals
                        daos = []
                        for hh in range(2):
                            h = 2 * hp + hh
                            dao = nrmp.tile([128, 512], f32, tag="dao", name="dao")
                            nc.vector.tensor_copy(dao[:], ppv[h][:])
                            daos.append(dao)
                        for hh in range(2):
                            dao = daos[hh]
                            rec = nrmp.tile([1, 512], f32, tag="rec", name="rec")
                            nc.vector.reciprocal_approx_fast(
                                rec[0:1, :], dao[0:1, :])
                            recr = nrmp.tile([1, 512], f32r, tag="recr", name="recr")
                            nc.vector.tensor_copy(recr[:], rec[:])

                            def mk_norm(qg=qg, c2=hp, off=64 * hh, recr=recr, dao=dao):
                                def emit():
                                    # PE-broadcast 1/denom across the 64 head
                                    # dims, then normalize into aot
                                    bc = ps_bc.tile([64, 512], f32, tag="bc", name="bc")
                                    nc.tensor.matmul(bc[:], ones_sb[:],
                                                     recr[:],
                                                     start=True, stop=True)
                                    nc.vector.tensor_mul(
                                        aot[c2][off:off + 64, 512 * qg:512 * (qg + 1)],
                                        dao[64:128, :], bc[:])
                                return emit
                            pending.append(mk_norm())
                # tail: the two remaining normalizes, then the last four
                # out-projection chunks (split across two DMA queues)
                while pending:
                    emit_pending_one()
                for i in range(4):
                    out_proj_chunk(12 + i, split=True)

    nc.compile()
    return nc


def _prep_inputs(x, token_positions, Wq, Wk, Wv, Wo):
    # even/odd interleave permutation within each head (for rotate-half RoPE)
    perm = np.concatenate([np.arange(0, DK, 2), np.arange(1, DK, 2)])

    pos = np.asarray(token_positions).astype(np.float32)
    angles = THETA ** (-np.arange(32, dtype=np.float32) / 32.0)
    ang = pos[:, None] * angles[None, :]          # [S, 32]
    cos32 = np.cos(ang).T.astype(np.float32)      # [32, S]
    sin32 = np.sin(ang).T.astype(np.float32)
    cos128 = np.concatenate([cos32, cos32, cos32, cos32], axis=0)
    sin128 = np.concatenate([-sin32, sin32, -sin32, sin32], axis=0)
    cos128 = np.ascontiguousarray(cos128).astype(np.float16)
    sin128 = np.ascontiguousarray(sin128).astype(np.float16)

    identity = _round_fp32r(np.eye(128, dtype=np.float32))

    Wq = np.asarray(Wq, dtype=np.float32)
    Wk = np.asarray(Wk, dtype=np.float32)
    Wv = np.asarray(Wv, dtype=np.float32)
    Wo = np.asarray(Wo, dtype=np.float32)
    x = np.asarray(x, dtype=np.float32)

    in_maps = []
    for c in range(N_CORES):
        b = c // 4
        h0 = (c % 4) * HL
        esl = slice(h0 * DK, (h0 + HL) * DK)
        wq_h = Wq[esl].reshape(HL, DK, D)[:, perm].reshape(EL, D)
        wk_h = Wk[esl].reshape(HL, DK, D)[:, perm].reshape(EL, D)
        wv_h = Wv[esl]
        bf = lambda a: np.ascontiguousarray(a, dtype=np.float32).astype(np.float16)
        in_maps.append({
            "xT": bf(x[b].T),
            "wq": bf(wq_h.T),
            "wk": bf(wk_h.T),
            "wv": bf(wv_h.T),
            "wo": bf(Wo[:, esl].T),
            "cosT": cos128,
            "sinT": sin128,
            "ident": identity,
        })
    return in_maps


def kernel(x, token_positions, Wq, Wk, Wv, Wo, _trace=False):
    from concourse.bass_utils import run_bass_kernel_spmd

    global _compiled
    if _compiled is None:
        _compiled = _build()
    in_maps = _prep_inputs(x, token_positions, Wq, Wk, Wv, Wo)
    res = run_bass_kernel_spmd(_compiled, in_maps, list(range(N_CORES)),
                               trace=_trace)
    parts = [res.results[c]["y"].astype(np.float64) for c in range(N_CORES)]
    out = np.empty((2, S, D), dtype=np.float32)
    out[0] = (parts[0] + parts[1] + parts[2] + parts[3]).astype(np.float32)
    out[1] = (parts[4] + parts[5] + parts[6] + parts[7]).astype(np.float32)
    if _trace:
        return out, res
    return out


# revision 15
# speedup vs baseline: 1.0145x; 1.0145x over previous
"""Trainium2 Bass kernel: multi-head self-attention with RoPE, causal mask.

Reference semantics (B=2, S=2048, D=1024, H=16, DK=64):
    q = rope(x @ Wq.T), k = rope(x @ Wk.T), v = x @ Wv.T   (per-head views)
    out = softmax(causal(q k^T / 8)) v ;  y = out @ Wo.T

Sharding over 8 cores: 2-way batch x 4-way heads (4 heads/core).
Each core computes a partial y [S, D] (its heads' contribution); host sums
the 4 partials per batch (device output is fp16, summed in fp64 on host).

On-device layout strategy (per core):
  - host passes xT = x[b].T [1024, 2048]; ALL 16-bit operands are fp16
    (better mantissa than bf16, same 1-cycle/row PE rate, 2x DVE rate)
  - Q/K projected ONCE; the rotate-half partner comes from a per-512-chunk
    SBUF-to-SBUF DMA block swap; rope = qa*cos (DVE) + qas*sin (gpsimd),
    add on DVE; PSUM evacuations ride the ACT engine
  - attention is processed HEAD-PAIR-OUTER: all of head-pair 0's q-groups
    run right after V + K(ec0) + Q(ec0) finish, with the K(ec1)/Q(ec1)
    projection chunks interleaved INTO that stream as PE filler (they use
    the out-projection's idle PSUM slot); head-pair 1 then runs with the
    output projection interleaved as filler
  - scores are computed TRANSPOSED (k on partitions, q on free); both
    heads of a pair write into ONE two-bank PSUM tile so a single Exp
    activation covers both (ACT instruction count halved; ACT binds)
  - V tiles are 128-wide blocks: ones column at col 0 (softmax denominator
    lands on PSUM partition 0 where the fast approx reciprocal works), V
    data at cols 64..127 (partition-base-64 aligned for the DVE multiply)
  - normalization: reciprocal_approx_fast + f32r copy + PE broadcast
    matmul, multiply deferred into the next stream's slack
"""

import sys

sys.path.insert(0, "/opt/trn_rl_repo")

import numpy as np
import ml_dtypes


S = 2048
D = 1024
NH = 16
DK = 64
HL = 4          # heads per core
EL = HL * DK    # 256 local e-dims
N_CORES = 8
THETA = 10000.0

_compiled = None


def _round_fp32r(x):
    # fp32r matmul operands must be pre-rounded to 11 mantissa bits (RNE)
    xi = np.ascontiguousarray(x, dtype=np.float32).view(np.uint32).astype(np.uint64)
    bias = ((xi >> 12) & 1) + (1 << 11) - 1
    return ((xi + bias) >> 12 << 12).astype(np.uint32).view(np.float32)


def _build():
    import concourse.bacc as bacc
    import concourse.tile as tile
    from concourse import mybir
    from concourse.alu_op_type import AluOpType

    dt = mybir.dt
    f32, f32r = dt.float32, dt.float32r
    f16 = dt.float16

    nc = bacc.Bacc("TRN2", target_bir_lowering=False, debug=False,
                   num_devices=N_CORES)

    xT = nc.dram_tensor("xT", [D, S], f16, kind="ExternalInput").ap()
    wq = nc.dram_tensor("wq", [D, EL], f16, kind="ExternalInput").ap()
    wk = nc.dram_tensor("wk", [D, EL], f16, kind="ExternalInput").ap()
    wv = nc.dram_tensor("wv", [D, EL], f16, kind="ExternalInput").ap()
    wo = nc.dram_tensor("wo", [EL, D], f16, kind="ExternalInput").ap()
    cosT = nc.dram_tensor("cosT", [128, S], f16, kind="ExternalInput").ap()
    sinT = nc.dram_tensor("sinT", [128, S], f16, kind="ExternalInput").ap()
    y = nc.dram_tensor("y", [S, D], f16, kind="ExternalOutput").ap()

    with tile.TileContext(nc) as tc:
        with tc.tile_pool(name="persist", bufs=1) as pp:
            # persistent SBUF tiles (live across both stages)
            qt = [pp.tile([128, S], f16, tag=f"qt{c}", name=f"qt{c}") for c in range(2)]
            ktz = [[pp.tile([128, S], f16, tag=f"ktz{c}{par}", name=f"ktz{c}{par}")
                   for par in range(2)] for c in range(2)]
            vh = [pp.tile([128, 16 * 128], f16, tag=f"v{h}", name=f"v{h}") for h in range(HL)]
            cos_sb = pp.tile([128, S], f16, tag="cos", name="cos")
            sin_sb = pp.tile([128, S], f16, tag="sin", name="sin")
            id_sb = pp.tile([128, 128], f32r, tag="ident", name="ident")
            xts = [pp.tile([128, S], f16, tag=f"xt{dc}", name=f"xt{dc}") for dc in range(8)]
            wks = [pp.tile([128, EL], f16, tag=f"wk{dc}", name=f"wk{dc}") for dc in range(8)]
            wqs = [pp.tile([128, EL], f16, tag=f"wq{dc}", name=f"wq{dc}") for dc in range(8)]

            def load_xt_slices(sg, split=1):
                for dc in range(8):
                    eng = nc.sync if dc % 2 == 0 else nc.scalar
                    p = 128 // split
                    for j in range(split):
                        eng.dma_start(
                            xts[dc][p * j:p * (j + 1), 512 * sg:512 * (sg + 1)],
                            xT[128 * dc + p * j:128 * dc + p * (j + 1),
                               512 * sg:512 * (sg + 1)])

            # rope chunk: evacuate PSUM proj (ACT), DMA block-swap to build
            # the rotate-half partner, cos/sin multiplies (DVE + gpsimd),
            # add into the destination K/Z or Q tile (DVE)
            def rope_chunk(ps, qa, qas, sg, is_k, ec, ropool, evac=None):
                sl = slice(512 * sg, 512 * (sg + 1))
                if evac is None:
                    nc.scalar.copy(qa[:, sl], ps[:])
                else:
                    evac(qa[:, sl], ps[:])
                for blk in range(2):
                    b0 = 64 * blk
                    nc.sync.dma_start(
                        qas[b0:b0 + 32, sl], qa[b0 + 32:b0 + 64, sl])
                    nc.sync.dma_start(
                        qas[b0 + 32:b0 + 64, sl], qa[b0:b0 + 32, sl])
                qc = ropool.tile([128, 512], f16, tag="qc", name="qc")
                qs = ropool.tile([128, 512], f16, tag="qs", name="qs")
                nc.vector.tensor_mul(qc[:], qa[:, sl], cos_sb[:, sl])
                nc.gpsimd.tensor_mul(qs[:], qas[:, sl], sin_sb[:, sl])
                if is_k:
                    # zero-padded K halves so score matmuls see K=128 rows
                    nc.vector.tensor_add(
                        ktz[ec][0][0:64, sl], qc[0:64, :], qs[0:64, :])
                    nc.vector.tensor_add(
                        ktz[ec][1][64:128, sl], qc[64:128, :], qs[64:128, :])
                else:
                    nc.vector.tensor_add(qt[ec][:, sl], qc[:], qs[:])

            # ======== stage A: V (all) + K/Q ec0 projections ========
            with tc.tile_pool(name="stagea", bufs=1) as sa, \
                 tc.tile_pool(name="qap", bufs=2) as qap, \
                 tc.tile_pool(name="ropa", bufs=3) as ropa, \
                 tc.tile_pool(name="vtp", bufs=2) as vtp, \
                 tc.tile_pool(name="pa", bufs=4, space="PSUM") as pap, \
                 tc.tile_pool(name="tr", bufs=2, space="PSUM") as trp:

                # warm up the PE clock-gate while input DMAs land; build
                # the transpose identity on-chip (no DMA needed)
                idf = sa.tile([128, 128], f32, tag="idf", name="idf")
                nc.vector.memset(idf[:], 1.0)
                idb = idf[:].bitcast(dt.bfloat16)   # values irrelevant
                wp = trp.tile([128, 512], f32, tag="warm", name="warm")
                for _ in range(12):
                    nc.tensor.matmul(wp[:, 0:256], idb[:, 0:128], idb[:],
                                     start=True, stop=True)
                nc.gpsimd.affine_select(
                    idf[:], idf[:], pattern=[[1, 128]],
                    compare_op=AluOpType.is_equal, fill=0.0,
                    base=0, channel_multiplier=-1)
                for _ in range(38):
                    nc.tensor.matmul(wp[:, 0:256], idb[:, 0:128], idb[:],
                                     start=True, stop=True)
                nc.vector.tensor_copy(id_sb[:], idf[:])

                load_xt_slices(0, split=2)
                wvs = [sa.tile([128, EL], f16, tag=f"w{dc}", name=f"w{dc}")
                       for dc in range(8)]
                for dc in range(8):
                    nc.sync.dma_start(wvs[dc][:], wv[128 * dc:128 * (dc + 1), :])
                load_xt_slices(1)
                load_xt_slices(2)
                load_xt_slices(3)
                for dc in range(8):
                    nc.sync.dma_start(wks[dc][:], wk[128 * dc:128 * (dc + 1), :])
                for half in range(2):
                    sl = slice(1024 * half, 1024 * (half + 1))
                    nc.scalar.dma_start(cos_sb[:, sl], cosT[:, sl])
                    nc.scalar.dma_start(sin_sb[:, sl], sinT[:, sl])
                for dc in range(8):
                    nc.scalar.dma_start(wqs[dc][:], wq[128 * dc:128 * (dc + 1), :])

                ones16 = sa.tile([128, 16], f32, tag="ones16", name="ones16")
                nc.vector.memset(ones16[:], 1.0)
                for c in range(2):
                    nc.gpsimd.memset(ktz[c][0][64:128, :], 0.0)
                    nc.gpsimd.memset(ktz[c][1][0:64, :], 0.0)

                # --- V projection (VT then PE-transpose into [s, dv]) ---
                for sg in range(4):
                    for ec in range(2):
                        ps = pap.tile([128, 512], f32, tag="pa", name="pa")
                        for dc in range(8):
                            nc.tensor.matmul(
                                ps[:],
                                wvs[dc][:, 128 * ec:128 * (ec + 1)],
                                xts[dc][:, 512 * sg:512 * (sg + 1)],
                                start=(dc == 0), stop=(dc == 7))
                        vt = vtp.tile([128, 512], f32r, tag="vt", name="vt")
                        nc.vector.tensor_copy(vt[:], ps[:])
                        tr = trp.tile([128, 512], f32r, tag="tr", name="tr")
                        for i in range(4):
                            nc.tensor.transpose(
                                tr[:, 128 * i:128 * (i + 1)],
                                vt[:, 128 * i:128 * (i + 1)], id_sb[:])
                        for i in range(4):
                            sc = 4 * sg + i
                            for hh in range(2):
                                h = 2 * ec + hh
                                nc.scalar.copy(
                                    vh[h][:, 128 * sc + 64:128 * sc + 128],
                                    tr[:, 128 * i + 64 * hh:128 * i + 64 * hh + 64])
                for h in range(HL):
                    ones_col = vh[h][:].rearrange(
                        "p (s c) -> p s c", c=128)[:, :, 0:1]
                    nc.vector.tensor_copy(ones_col, ones16[:].rearrange("p (s c) -> p s c", c=1))

                # --- K ec0 then Q ec0 (head pair 0's tensors) ---
                for is_k, ws in [(True, wks), (False, wqs)]:
                    qa = qap.tile([128, S], f16, tag="qa", name="qa")
                    qas = qap.tile([128, S], f16, tag="qas", name="qas")
                    for sg in range(4):
                        ps = pap.tile([128, 512], f32, tag="pa", name="pa")
                        for dc in range(8):
                            nc.tensor.matmul(
                                ps[:],
                                ws[dc][:, 0:128],
                                xts[dc][:, 512 * sg:512 * (sg + 1)],
                                start=(dc == 0), stop=(dc == 7))
                        rope_chunk(ps, qa, qas, sg, is_k, 0, ropa)

            # ======== stage B: attention (head-pair outer) + out-proj ========
            with tc.tile_pool(name="pb", bufs=1) as pb, \
                 tc.tile_pool(name="ptp", bufs=8) as ptp, \
                 tc.tile_pool(name="nrm", bufs=4) as nrmp, \
                 tc.tile_pool(name="ysb", bufs=2) as ysbp, \
                 tc.tile_pool(name="kqp", bufs=2) as kqp, \
                 tc.tile_pool(name="ropb", bufs=3) as ropb, \
                 tc.tile_pool(name="ps_s", bufs=2, space="PSUM") as ps_s, \
                 tc.tile_pool(name="ps_pv", bufs=2, space="PSUM") as ps_pv, \
                 tc.tile_pool(name="ps_y", bufs=1, space="PSUM") as ps_y, \
                 tc.tile_pool(name="ps_bc", bufs=1, space="PSUM") as ps_bc:

                aot = [pb.tile([128, S], f16, tag=f"aot{c}", name=f"aot{c}") for c in range(2)]
                wo_sb = [pb.tile([128, D], f16, tag=f"wo{c}", name=f"wo{c}") for c in range(2)]
                ones_sb = pb.tile([1, 64], f32r, tag="ones", name="ones")
                onesf = pb.tile([1, 64], f32, tag="onesf", name="onesf")
                nc.vector.memset(onesf[:], 1.0)
                nc.vector.tensor_copy(ones_sb[:], onesf[:])
                for c in range(2):
                    for half in range(2):
                        sl = slice(512 * half, 512 * (half + 1))
                        nc.scalar.dma_start(wo_sb[c][:, sl],
                                            wo[128 * c:128 * (c + 1), sl])

                # ec1 projection chunks, emitted as PE filler inside head-
                # pair 0's attention stream (use the out-projection's PSUM
                # slot, which is idle until head-pair 1)
                kq_tiles = {}
                for is_k in (True, False):
                    kq_tiles[is_k] = (
                        kqp.tile([128, S], f16, tag="qa", name="qa"),
                        kqp.tile([128, S], f16, tag="qas", name="qas"))

                # ec1 projection chunks as ~0.9us PE pieces: half the
                # accumulation chain per piece (other-bank matmuls may
                # interleave inside an open PSUM accumulation group)
                def mk_proj_pieces(is_k, sg):
                    box = {}

                    def piece1():
                        ws = wks if is_k else wqs
                        box["ps"] = ps_y.tile([128, 512], f32, tag="yp", name="yp")
                        for dc in range(4):
                            nc.tensor.matmul(
                                box["ps"][:],
                                ws[dc][:, 128:256],
                                xts[dc][:, 512 * sg:512 * (sg + 1)],
                                start=(dc == 0), stop=False)

                    def piece2():
                        ws = wks if is_k else wqs
                        qa, qas = kq_tiles[is_k]
                        for dc in range(4, 8):
                            nc.tensor.matmul(
                                box["ps"][:],
                                ws[dc][:, 128:256],
                                xts[dc][:, 512 * sg:512 * (sg + 1)],
                                start=False, stop=(dc == 7))
                        rope_chunk(ps=box["ps"], qa=qa, qas=qas, sg=sg,
                                   is_k=is_k, ec=1, ropool=ropb,
                                   evac=nc.vector.tensor_copy)
                    return [piece1, piece2]

                filler_q = []
                for is_k in (True, False):
                    for sg in range(4):
                        filler_q += mk_proj_pieces(is_k, sg)

                def out_proj_eg(sc, eg, ysb, split=False):
                    yp = ps_y.tile([128, 512], f32, tag="yp", name="yp")
                    for c2 in range(2):
                        nc.tensor.matmul(
                            yp[:],
                            aot[c2][:, 128 * sc:128 * (sc + 1)],
                            wo_sb[c2][:, 512 * eg:512 * (eg + 1)],
                            start=(c2 == 0), stop=(c2 == 1))
                    nc.vector.tensor_copy(
                        ysb[:, 512 * eg:512 * (eg + 1)], yp[:])
                    if eg == 1:
                        for half in range(2):
                            sl = slice(512 * half, 512 * (half + 1))
                            if split:
                                nc.sync.dma_start(
                                    y[128 * sc:128 * sc + 64, sl], ysb[0:64, sl])
                                nc.scalar.dma_start(
                                    y[128 * sc + 64:128 * (sc + 1), sl], ysb[64:128, sl])
                            else:
                                nc.sync.dma_start(
                                    y[128 * sc:128 * (sc + 1), sl], ysb[:, sl])

                def out_proj_pieces(sc, split=False):
                    box = {}

                    def p1():
                        box["ysb"] = ysbp.tile([128, D], f16, tag="ysb", name="ysb")
                        out_proj_eg(sc, 0, box["ysb"], split)

                    def p2():
                        out_proj_eg(sc, 1, box["ysb"], split)
                    return [p1, p2]

                def out_proj_chunk(sc, split=False):
                    for p in out_proj_pieces(sc, split):
                        p()

                pending = []   # deferred normalize closures

                def emit_pending_one():
                    if pending:
                        pending.pop(0)()

                SKEW = 3
                for hp in range(2):
                    for qg in range(4):
                        n_kc = 4 * qg + 4
                        # flush the previous stream's two normalizes early,
                        # then enqueue that q-group's out-projection pieces
                        # as per-kc PE filler
                        norm_at = {} if (hp, qg) == (0, 0) else {1: 2}
                        if hp == 1 and qg >= 1:
                            for sc in range(4 * (qg - 1), 4 * qg):
                                filler_q += out_proj_pieces(sc)
                        ppv = {}
                        for hh in range(2):
                            h = 2 * hp + hh
                            ppv[h] = ps_pv.tile([128, 512], f32, tag="ppv", name="ppv")
                        ptq = {}
                        for kc in range(n_kc + SKEW):
                            for _ in range(norm_at.get(kc, 0)):
                                emit_pending_one()
                            if kc >= 2 and filler_q:
                                filler_q.pop(0)()
                            # PV first: keeps queued work ahead of a score
                            # matmul that may block on PSUM reuse
                            kcp = kc - SKEW
                            if kcp >= 0:
                                ptv2, q0v = ptq.pop(kcp)
                                for hh in range(2):
                                    h = 2 * hp + hh
                                    nc.tensor.matmul(
                                        ppv[h][:, q0v:512],
                                        vh[h][:, 128 * kcp:128 * kcp + 128],
                                        ptv2[:, 512 * hh + q0v:512 * (hh + 1)],
                                        start=(kcp == 0), stop=(kcp == n_kc - 1))
                            if kc < n_kc:
                                # diagonal tiles only need q >= k
                                r = kc - 4 * qg
                                q0 = 128 * r if r > 0 else 0
                                qsl = slice(512 * qg + q0, 512 * (qg + 1))
                                ps2 = ps_s.tile([128, 1024], f32, tag="ps", name="ps")
                                for hh in range(2):
                                    nc.tensor.matmul(
                                        ps2[:, 512 * hh + q0:512 * (hh + 1)],
                                        ktz[hp][hh][:, 128 * kc:128 * (kc + 1)],
                                        qt[hp][:, qsl],
                                        start=True, stop=True)
                                pt = ptp.tile([128, 1024], f16, tag="pt", name="pt")
                                psv = ps2[:].rearrange("p (h q) -> p h q", h=2)[:, :, q0:512]
                                ptv = pt[:].rearrange("p (h q) -> p h q", h=2)[:, :, q0:512]
                                nc.scalar.activation(
                                    ptv, psv,
                                    mybir.ActivationFunctionType.Exp,
                                    scale=0.125)
                                if r >= 0:
                                    for hh in range(2):
                                        nc.gpsimd.affine_select(
                                            pt[:, 512 * hh + q0:512 * (hh + 1)],
                                            pt[:, 512 * hh + q0:512 * (hh + 1)],
                                            pattern=[[1, 512 - q0]],
                                            compare_op=AluOpType.is_ge, fill=0.0,
                                            base=512 * qg + q0 - 128 * kc,
                                            channel_multiplier=-1)
                                ptq[kc] = (pt, q0)
                        # evacuate ppv fast: BOTH attn-out+denom copies first
                        # (they gate PSUM reuse), then the cheap reciprocals
                        daos = []
                        for hh in range(2):
                            h = 2 * hp + hh
                            dao = nrmp.tile([128, 512], f32, tag="dao", name="dao")
                            nc.vector.tensor_copy(dao[:], ppv[h][:])
                            daos.append(dao)
                        for hh in range(2):
                            dao = daos[hh]
                            rec = nrmp.tile([1, 512], f32, tag="rec", name="rec")
                            nc.vector.reciprocal_approx_fast(
                                rec[0:1, :], dao[0:1, :])
                            recr = nrmp.tile([1, 512], f32r, tag="recr", name="recr")
                            nc.vector.tensor_copy(recr[:], rec[:])

                            def mk_norm(qg=qg, c2=hp, off=64 * hh, recr=recr, dao=dao):
                                def emit():
                                    # PE-broadcast 1/denom across the 64 head
                                    # dims, then normalize into aot
                                    bc = ps_bc.tile([64, 512], f32, tag="bc", name="bc")
                                    nc.tensor.matmul(bc[:], ones_sb[:],
                                                     recr[:],
                                                     start=True, stop=True)
                                    nc.vector.tensor_mul(
                                        aot[c2][off:off + 64, 512 * qg:512 * (qg + 1)],
                                        dao[64:128, :], bc[:])
                                return emit
                            pending.append(mk_norm())
                # tail: the two remaining normalizes, then the last four
                # out-projection chunks (split across two DMA queues)
                while pending:
                    emit_pending_one()
                for i in range(4):
                    out_proj_chunk(12 + i, split=True)

    nc.compile()
    return nc


def _prep_inputs(x, token_positions, Wq, Wk, Wv, Wo):
    # even/odd interleave permutation within each head (for rotate-half RoPE)
    perm = np.concatenate([np.arange(0, DK, 2), np.arange(1, DK, 2)])

    pos = np.asarray(token_positions).astype(np.float32)
    angles = THETA ** (-np.arange(32, dtype=np.float32) / 32.0)
    ang = pos[:, None] * angles[None, :]          # [S, 32]
    cos32 = np.cos(ang).T.astype(np.float32)      # [32, S]
    sin32 = np.sin(ang).T.astype(np.float32)
    cos128 = np.concatenate([cos32, cos32, cos32, cos32], axis=0)
    sin128 = np.concatenate([-sin32, sin32, -sin32, sin32], axis=0)
    cos128 = np.ascontiguousarray(cos128).astype(np.float16)
    sin128 = np.ascontiguousarray(sin128).astype(np.float16)

    identity = _round_fp32r(np.eye(128, dtype=np.float32))

    Wq = np.asarray(Wq, dtype=np.float32)
    Wk = np.asarray(Wk, dtype=np.float32)
    Wv = np.asarray(Wv, dtype=np.float32)
    Wo = np.asarray(Wo, dtype=np.float32)
    x = np.asarray(x, dtype=np.float32)

    in_maps = []
    for c in range(N_CORES):
        b = c // 4
        h0 = (c % 4) * HL
        esl = slice(h0 * DK, (h0 + HL) * DK)
        wq_h = Wq[esl].reshape(HL, DK, D)[:, perm].reshape(EL, D)
        wk_h = Wk[esl].reshape(HL, DK, D)[:, perm].reshape(EL, D)
        wv_h = Wv[esl]
        bf = lambda a: np.ascontiguousarray(a, dtype=np.float32).astype(np.float16)
        in_maps.append({
            "xT": bf(x[b].T),
            "wq": bf(wq_h.T),
            "wk": bf(wk_h.T),
            "wv": bf(wv_h.T),
            "wo": bf(Wo[:, esl].T),
            "cosT": cos128,
            "sinT": sin128,
            "ident": identity,
        })
    return in_maps


def kernel(x, token_positions, Wq, Wk, Wv, Wo, _trace=False):
    from concourse.bass_utils import run_bass_kernel_spmd

    global _compiled
    if _compiled is None:
        _compiled = _build()
    in_maps = _prep_inputs(x, token_positions, Wq, Wk, Wv, Wo)
    res = run_bass_kernel_spmd(_compiled, in_maps, list(range(N_CORES)),
                               trace=_trace)
    parts = [res.results[c]["y"].astype(np.float64) for c in range(N_CORES)]
    out = np.empty((2, S, D), dtype=np.float32)
    out[0] = (parts[0] + parts[1] + parts[2] + parts[3]).astype(np.float32)
    out[1] = (parts[4] + parts[5] + parts[6] + parts[7]).astype(np.float32)
    if _trace:
        return out, res
    return out


# revision 16
# speedup vs baseline: 1.0211x; 1.0065x over previous
"""Trainium2 Bass kernel: multi-head self-attention with RoPE, causal mask.

Reference semantics (B=2, S=2048, D=1024, H=16, DK=64):
    q = rope(x @ Wq.T), k = rope(x @ Wk.T), v = x @ Wv.T   (per-head views)
    out = softmax(causal(q k^T / 8)) v ;  y = out @ Wo.T

Sharding over 8 cores: 2-way batch x 4-way heads (4 heads/core).
Each core computes a partial y [S, D] (its heads' contribution); host sums
the 4 partials per batch (device output is fp16, summed in fp64 on host).

On-device layout strategy (per core):
  - host passes xT = x[b].T [1024, 2048]; ALL 16-bit operands are fp16
    (better mantissa than bf16, same 1-cycle/row PE rate, 2x DVE rate)
  - Q/K projected ONCE; the rotate-half partner comes from a per-512-chunk
    SBUF-to-SBUF DMA block swap; rope = qa*cos (DVE) + qas*sin (gpsimd),
    add on DVE; PSUM evacuations ride the ACT engine
  - attention is processed HEAD-PAIR-OUTER: all of head-pair 0's q-groups
    run right after V + K(ec0) + Q(ec0) finish, with the K(ec1)/Q(ec1)
    projection chunks interleaved INTO that stream as PE filler (they use
    the out-projection's idle PSUM slot); head-pair 1 then runs with the
    output projection interleaved as filler
  - scores are computed TRANSPOSED (k on partitions, q on free); both
    heads of a pair write into ONE two-bank PSUM tile so a single Exp
    activation covers both (ACT instruction count halved; ACT binds)
  - V tiles are 128-wide blocks: ones column at col 0 (softmax denominator
    lands on PSUM partition 0 where the fast approx reciprocal works), V
    data at cols 64..127 (partition-base-64 aligned for the DVE multiply)
  - normalization: reciprocal_approx_fast + f32r copy + PE broadcast
    matmul, multiply deferred into the next stream's slack
"""

import sys

sys.path.insert(0, "/opt/trn_rl_repo")

import numpy as np
import ml_dtypes


S = 2048
D = 1024
NH = 16
DK = 64
HL = 4          # heads per core
EL = HL * DK    # 256 local e-dims
N_CORES = 8
THETA = 10000.0

_compiled = None


def _round_fp32r(x):
    # fp32r matmul operands must be pre-rounded to 11 mantissa bits (RNE)
    xi = np.ascontiguousarray(x, dtype=np.float32).view(np.uint32).astype(np.uint64)
    bias = ((xi >> 12) & 1) + (1 << 11) - 1
    return ((xi + bias) >> 12 << 12).astype(np.uint32).view(np.float32)


def _build():
    import concourse.bacc as bacc
    import concourse.tile as tile
    from concourse import mybir
    from concourse.alu_op_type import AluOpType

    dt = mybir.dt
    f32, f32r = dt.float32, dt.float32r
    f16 = dt.float16

    nc = bacc.Bacc("TRN2", target_bir_lowering=False, debug=False,
                   num_devices=N_CORES)

    xT = nc.dram_tensor("xT", [D, S], f16, kind="ExternalInput").ap()
    wq = nc.dram_tensor("wq", [D, EL], f16, kind="ExternalInput").ap()
    wk = nc.dram_tensor("wk", [D, EL], f16, kind="ExternalInput").ap()
    wv = nc.dram_tensor("wv", [D, EL], f16, kind="ExternalInput").ap()
    wo = nc.dram_tensor("wo", [EL, D], f16, kind="ExternalInput").ap()
    cosT = nc.dram_tensor("cosT", [128, S], f16, kind="ExternalInput").ap()
    sinT = nc.dram_tensor("sinT", [128, S], f16, kind="ExternalInput").ap()
    y = nc.dram_tensor("y", [S, D], f16, kind="ExternalOutput").ap()

    with tile.TileContext(nc) as tc:
        with tc.tile_pool(name="persist", bufs=1) as pp:
            # persistent SBUF tiles (live across both stages)
            qt = [pp.tile([128, S], f16, tag=f"qt{c}", name=f"qt{c}") for c in range(2)]
            ktz = [[pp.tile([128, S], f16, tag=f"ktz{c}{par}", name=f"ktz{c}{par}")
                   for par in range(2)] for c in range(2)]
            vh = [pp.tile([128, 16 * 128], f16, tag=f"v{h}", name=f"v{h}") for h in range(HL)]
            cos_sb = pp.tile([128, S], f16, tag="cos", name="cos")
            sin_sb = pp.tile([128, S], f16, tag="sin", name="sin")
            id_sb = pp.tile([128, 128], f32r, tag="ident", name="ident")
            xts = [pp.tile([128, S], f16, tag=f"xt{dc}", name=f"xt{dc}") for dc in range(8)]
            wks = [pp.tile([128, EL], f16, tag=f"wk{dc}", name=f"wk{dc}") for dc in range(8)]
            wqs = [pp.tile([128, EL], f16, tag=f"wq{dc}", name=f"wq{dc}") for dc in range(8)]

            def load_xt_slices(sg, split=1):
                for dc in range(8):
                    eng = nc.sync if dc % 2 == 0 else nc.scalar
                    p = 128 // split
                    for j in range(split):
                        eng.dma_start(
                            xts[dc][p * j:p * (j + 1), 512 * sg:512 * (sg + 1)],
                            xT[128 * dc + p * j:128 * dc + p * (j + 1),
                               512 * sg:512 * (sg + 1)])

            # rope chunk: evacuate PSUM proj (ACT), DMA block-swap to build
            # the rotate-half partner, cos/sin multiplies (DVE + gpsimd),
            # add into the destination K/Z or Q tile (DVE)
            def rope_chunk(ps, qa, qas, sg, is_k, ec, ropool, evac=None):
                sl = slice(512 * sg, 512 * (sg + 1))
                if evac is None:
                    nc.scalar.copy(qa[:, sl], ps[:])
                else:
                    evac(qa[:, sl], ps[:])
                for blk in range(2):
                    b0 = 64 * blk
                    nc.sync.dma_start(
                        qas[b0:b0 + 32, sl], qa[b0 + 32:b0 + 64, sl])
                    nc.sync.dma_start(
                        qas[b0 + 32:b0 + 64, sl], qa[b0:b0 + 32, sl])
                qc = ropool.tile([128, 512], f16, tag="qc", name="qc")
                qs = ropool.tile([128, 512], f16, tag="qs", name="qs")
                nc.vector.tensor_mul(qc[:], qa[:, sl], cos_sb[:, sl])
                nc.gpsimd.tensor_mul(qs[:], qas[:, sl], sin_sb[:, sl])
                if is_k:
                    # zero-padded K halves so score matmuls see K=128 rows
                    nc.vector.tensor_add(
                        ktz[ec][0][0:64, sl], qc[0:64, :], qs[0:64, :])
                    nc.vector.tensor_add(
                        ktz[ec][1][64:128, sl], qc[64:128, :], qs[64:128, :])
                else:
                    nc.vector.tensor_add(qt[ec][:, sl], qc[:], qs[:])

            # ======== stage A: V (all) + K/Q ec0 projections ========
            with tc.tile_pool(name="stagea", bufs=1) as sa, \
                 tc.tile_pool(name="qap", bufs=2) as qap, \
                 tc.tile_pool(name="ropa", bufs=3) as ropa, \
                 tc.tile_pool(name="vtp", bufs=2) as vtp, \
                 tc.tile_pool(name="pa", bufs=4, space="PSUM") as pap, \
                 tc.tile_pool(name="tr", bufs=2, space="PSUM") as trp:

                # warm up the PE clock-gate while input DMAs land; build
                # the transpose identity on-chip (no DMA needed)
                idf = sa.tile([128, 128], f32, tag="idf", name="idf")
                nc.vector.memset(idf[:], 1.0)
                idb = idf[:].bitcast(dt.bfloat16)   # values irrelevant
                wp = trp.tile([128, 512], f32, tag="warm", name="warm")
                for _ in range(12):
                    nc.tensor.matmul(wp[:, 0:256], idb[:, 0:128], idb[:],
                                     start=True, stop=True)
                nc.gpsimd.affine_select(
                    idf[:], idf[:], pattern=[[1, 128]],
                    compare_op=AluOpType.is_equal, fill=0.0,
                    base=0, channel_multiplier=-1)
                for _ in range(38):
                    nc.tensor.matmul(wp[:, 0:256], idb[:, 0:128], idb[:],
                                     start=True, stop=True)
                nc.vector.tensor_copy(id_sb[:], idf[:])

                load_xt_slices(0, split=2)
                wvs = [sa.tile([128, EL], f16, tag=f"w{dc}", name=f"w{dc}")
                       for dc in range(8)]
                for dc in range(8):
                    nc.sync.dma_start(wvs[dc][:], wv[128 * dc:128 * (dc + 1), :])
                load_xt_slices(1)
                load_xt_slices(2)
                load_xt_slices(3)
                for dc in range(8):
                    nc.sync.dma_start(wks[dc][:], wk[128 * dc:128 * (dc + 1), :])
                for half in range(2):
                    sl = slice(1024 * half, 1024 * (half + 1))
                    nc.scalar.dma_start(cos_sb[:, sl], cosT[:, sl])
                    nc.scalar.dma_start(sin_sb[:, sl], sinT[:, sl])
                for dc in range(8):
                    nc.scalar.dma_start(wqs[dc][:], wq[128 * dc:128 * (dc + 1), :])

                ones16 = sa.tile([128, 16], f32, tag="ones16", name="ones16")
                nc.vector.memset(ones16[:], 1.0)
                for c in range(2):
                    nc.gpsimd.memset(ktz[c][0][64:128, :], 0.0)
                    nc.gpsimd.memset(ktz[c][1][0:64, :], 0.0)

                # --- V projection (VT then PE-transpose into [s, dv]) ---
                for sg in range(4):
                    for ec in range(2):
                        ps = pap.tile([128, 512], f32, tag="pa", name="pa")
                        for dc in range(8):
                            nc.tensor.matmul(
                                ps[:],
                                wvs[dc][:, 128 * ec:128 * (ec + 1)],
                                xts[dc][:, 512 * sg:512 * (sg + 1)],
                                start=(dc == 0), stop=(dc == 7))
                        vt = vtp.tile([128, 512], f32r, tag="vt", name="vt")
                        nc.vector.tensor_copy(vt[:], ps[:])
                        tr = trp.tile([128, 512], f32r, tag="tr", name="tr")
                        for i in range(4):
                            nc.tensor.transpose(
                                tr[:, 128 * i:128 * (i + 1)],
                                vt[:, 128 * i:128 * (i + 1)], id_sb[:])
                        for i in range(4):
                            sc = 4 * sg + i
                            for hh in range(2):
                                h = 2 * ec + hh
                                nc.scalar.copy(
                                    vh[h][:, 128 * sc + 64:128 * sc + 128],
                                    tr[:, 128 * i + 64 * hh:128 * i + 64 * hh + 64])
                for h in range(HL):
                    ones_col = vh[h][:].rearrange(
                        "p (s c) -> p s c", c=128)[:, :, 0:1]
                    nc.vector.tensor_copy(ones_col, ones16[:].rearrange("p (s c) -> p s c", c=1))

                # --- K ec0 then Q ec0 (head pair 0's tensors) ---
                for is_k, ws in [(True, wks), (False, wqs)]:
                    qa = qap.tile([128, S], f16, tag="qa", name="qa")
                    qas = qap.tile([128, S], f16, tag="qas", name="qas")
                    for sg in range(4):
                        ps = pap.tile([128, 512], f32, tag="pa", name="pa")
                        for dc in range(8):
                            nc.tensor.matmul(
                                ps[:],
                                ws[dc][:, 0:128],
                                xts[dc][:, 512 * sg:512 * (sg + 1)],
                                start=(dc == 0), stop=(dc == 7))
                        rope_chunk(ps, qa, qas, sg, is_k, 0, ropa)

            # ======== stage B: attention (head-pair outer) + out-proj ========
            with tc.tile_pool(name="pb", bufs=1) as pb, \
                 tc.tile_pool(name="ptp", bufs=8) as ptp, \
                 tc.tile_pool(name="nrm", bufs=4) as nrmp, \
                 tc.tile_pool(name="ysb", bufs=2) as ysbp, \
                 tc.tile_pool(name="kqp", bufs=2) as kqp, \
                 tc.tile_pool(name="ropb", bufs=3) as ropb, \
                 tc.tile_pool(name="ps_s", bufs=2, space="PSUM") as ps_s, \
                 tc.tile_pool(name="ps_pv", bufs=2, space="PSUM") as ps_pv, \
                 tc.tile_pool(name="ps_y", bufs=1, space="PSUM") as ps_y, \
                 tc.tile_pool(name="ps_bc", bufs=1, space="PSUM") as ps_bc:

                aot = [pb.tile([128, S], f16, tag=f"aot{c}", name=f"aot{c}") for c in range(2)]
                wo_sb = [pb.tile([128, D], f16, tag=f"wo{c}", name=f"wo{c}") for c in range(2)]
                ones_sb = pb.tile([1, 64], f32r, tag="ones", name="ones")
                onesf = pb.tile([1, 64], f32, tag="onesf", name="onesf")
                nc.vector.memset(onesf[:], 1.0)
                nc.vector.tensor_copy(ones_sb[:], onesf[:])
                for c in range(2):
                    for half in range(2):
                        sl = slice(512 * half, 512 * (half + 1))
                        nc.scalar.dma_start(wo_sb[c][:, sl],
                                            wo[128 * c:128 * (c + 1), sl])

                # ec1 projection chunks, emitted as PE filler inside head-
                # pair 0's attention stream (use the out-projection's PSUM
                # slot, which is idle until head-pair 1)
                kq_tiles = {}
                for is_k in (True, False):
                    kq_tiles[is_k] = (
                        kqp.tile([128, S], f16, tag="qa", name="qa"),
                        kqp.tile([128, S], f16, tag="qas", name="qas"))

                # ec1 projection chunks as ~0.9us PE pieces: half the
                # accumulation chain per piece (other-bank matmuls may
                # interleave inside an open PSUM accumulation group)
                def mk_proj_pieces(is_k, sg):
                    box = {}

                    def piece1():
                        ws = wks if is_k else wqs
                        box["ps"] = ps_y.tile([128, 512], f32, tag="yp", name="yp")
                        for dc in range(4):
                            nc.tensor.matmul(
                                box["ps"][:],
                                ws[dc][:, 128:256],
                                xts[dc][:, 512 * sg:512 * (sg + 1)],
                                start=(dc == 0), stop=False)

                    def piece2():
                        ws = wks if is_k else wqs
                        qa, qas = kq_tiles[is_k]
                        for dc in range(4, 8):
                            nc.tensor.matmul(
                                box["ps"][:],
                                ws[dc][:, 128:256],
                                xts[dc][:, 512 * sg:512 * (sg + 1)],
                                start=False, stop=(dc == 7))
                        rope_chunk(ps=box["ps"], qa=qa, qas=qas, sg=sg,
                                   is_k=is_k, ec=1, ropool=ropb,
                                   evac=nc.vector.tensor_copy)
                    return [piece1, piece2]

                filler_q = []
                for is_k in (True, False):
                    for sg in range(4):
                        filler_q += mk_proj_pieces(is_k, sg)

                def out_proj_eg(sc, eg, ysb, split=False):
                    yp = ps_y.tile([128, 512], f32, tag="yp", name="yp")
                    for c2 in range(2):
                        nc.tensor.matmul(
                            yp[:],
                            aot[c2][:, 128 * sc:128 * (sc + 1)],
                            wo_sb[c2][:, 512 * eg:512 * (eg + 1)],
                            start=(c2 == 0), stop=(c2 == 1))
                    nc.vector.tensor_copy(
                        ysb[:, 512 * eg:512 * (eg + 1)], yp[:])
                    if eg == 1:
                        for half in range(2):
                            sl = slice(512 * half, 512 * (half + 1))
                            if split:
                                nc.sync.dma_start(
                                    y[128 * sc:128 * sc + 64, sl], ysb[0:64, sl])
                                nc.scalar.dma_start(
                                    y[128 * sc + 64:128 * (sc + 1), sl], ysb[64:128, sl])
                            else:
                                nc.sync.dma_start(
                                    y[128 * sc:128 * (sc + 1), sl], ysb[:, sl])

                def out_proj_pieces(sc, split=False):
                    box = {}

                    def p1():
                        box["ysb"] = ysbp.tile([128, D], f16, tag="ysb", name="ysb")
                        out_proj_eg(sc, 0, box["ysb"], split)

                    def p2():
                        out_proj_eg(sc, 1, box["ysb"], split)
                    return [p1, p2]

                def out_proj_chunk(sc, split=False):
                    for p in out_proj_pieces(sc, split):
                        p()

                pending = []   # deferred normalize closures

                def emit_pending_one():
                    if pending:
                        pending.pop(0)()

                SKEW = 3
                for hp in range(2):
                    for qg in range(4):
                        n_kc = 4 * qg + 4
                        # flush the previous stream's two normalizes early,
                        # then enqueue that q-group's out-projection pieces
                        # as per-kc PE filler
                        norm_at = {} if (hp, qg) == (0, 0) else {1: 2}
                        if hp == 1 and qg >= 1:
                            for sc in range(4 * (qg - 1), 4 * qg):
                                filler_q += out_proj_pieces(sc)
                        ppv = {}
                        for hh in range(2):
                            h = 2 * hp + hh
                            ppv[h] = ps_pv.tile([128, 512], f32, tag="ppv", name="ppv")
                        ptq = {}
                        for kc in range(n_kc + SKEW):
                            for _ in range(norm_at.get(kc, 0)):
                                emit_pending_one()
                            if kc >= 2 and filler_q:
                                filler_q.pop(0)()
                            # PV first: keeps queued work ahead of a score
                            # matmul that may block on PSUM reuse
                            kcp = kc - SKEW
                            if kcp >= 0:
                                ptv2, q0v = ptq.pop(kcp)
                                for hh in range(2):
                                    h = 2 * hp + hh
                                    nc.tensor.matmul(
                                        ppv[h][:, q0v:512],
                                        vh[h][:, 128 * kcp:128 * kcp + 128],
                                        ptv2[:, 512 * hh + q0v:512 * (hh + 1)],
                                        start=(kcp == 0), stop=(kcp == n_kc - 1))
                            if kc < n_kc:
                                # diagonal tiles only need q >= k
                                r = kc - 4 * qg
                                q0 = 128 * r if r > 0 else 0
                                qsl = slice(512 * qg + q0, 512 * (qg + 1))
                                ps2 = ps_s.tile([128, 1024], f32, tag="ps", name="ps")
                                for hh in range(2):
                                    nc.tensor.matmul(
                                        ps2[:, 512 * hh + q0:512 * (hh + 1)],
                                        ktz[hp][hh][:, 128 * kc:128 * (kc + 1)],
                                        qt[hp][:, qsl],
                                        start=True, stop=True)
                                pt = ptp.tile([128, 1024], f16, tag="pt", name="pt")
                                psv = ps2[:].rearrange("p (h q) -> p h q", h=2)[:, :, q0:512]
                                ptv = pt[:].rearrange("p (h q) -> p h q", h=2)[:, :, q0:512]
                                nc.scalar.activation(
                                    ptv, psv,
                                    mybir.ActivationFunctionType.Exp,
                                    scale=0.125)
                                if r >= 0:
                                    for hh in range(2):
                                        nc.gpsimd.affine_select(
                                            pt[:, 512 * hh + q0:512 * (hh + 1)],
                                            pt[:, 512 * hh + q0:512 * (hh + 1)],
                                            pattern=[[1, 512 - q0]],
                                            compare_op=AluOpType.is_ge, fill=0.0,
                                            base=512 * qg + q0 - 128 * kc,
                                            channel_multiplier=-1)
                                ptq[kc] = (pt, q0)
                        # evacuate ppv fast: BOTH attn-out+denom copies first
                        # (they gate PSUM reuse), then the cheap reciprocals
                        daos = []
                        for hh in range(2):
                            h = 2 * hp + hh
                            dao = nrmp.tile([128, 512], f32, tag="dao", name="dao")
                            nc.vector.tensor_copy(dao[:], ppv[h][:])
                            daos.append(dao)
                        for hh in range(2):
                            dao = daos[hh]
                            rec = nrmp.tile([1, 512], f32, tag="rec", name="rec")
                            nc.vector.reciprocal_approx_fast(
                                rec[0:1, :], dao[0:1, :])
                            recr = nrmp.tile([1, 512], f32r, tag="recr", name="recr")
                            nc.vector.tensor_copy(recr[:], rec[:])

                            def mk_norm(qg=qg, c2=hp, off=64 * hh, recr=recr, dao=dao):
                                def emit():
                                    # PE-broadcast 1/denom across the 64 head
                                    # dims, then normalize into aot
                                    bc = ps_bc.tile([64, 512], f32, tag="bc", name="bc")
                                    nc.tensor.matmul(bc[:], ones_sb[:],
                                                     recr[:],
                                                     start=True, stop=True)
                                    nc.vector.tensor_mul(
                                        aot[c2][off:off + 64, 512 * qg:512 * (qg + 1)],
                                        dao[64:128, :], bc[:])
                                return emit
                            pending.append(mk_norm())
                # tail: the two remaining normalizes, then the last four
                # out-projection chunks.  The wide score-PSUM pool is idle
                # now — run each chunk out of one [128,1024] tile (two yp
                # slots) so consecutive chunks never serialize on PSUM
                # reuse, evacuate with ONE wide DVE copy, and split the
                # writeback DMAs across both queues.
                while pending:
                    emit_pending_one()
                for i in range(4):
                    sc = 12 + i
                    ps2 = ps_s.tile([128, 1024], f32, tag="ps", name="ps")
                    for eg in range(2):
                        for c2 in range(2):
                            nc.tensor.matmul(
                                ps2[:, 512 * eg:512 * (eg + 1)],
                                aot[c2][:, 128 * sc:128 * (sc + 1)],
                                wo_sb[c2][:, 512 * eg:512 * (eg + 1)],
                                start=(c2 == 0), stop=(c2 == 1))
                    ysb = ysbp.tile([128, D], f16, tag="ysb", name="ysb")
                    nc.vector.tensor_copy(ysb[:], ps2[:])
                    for half in range(2):
                        sl = slice(512 * half, 512 * (half + 1))
                        nc.sync.dma_start(
                            y[128 * sc:128 * sc + 64, sl], ysb[0:64, sl])
                        nc.scalar.dma_start(
                            y[128 * sc + 64:128 * (sc + 1), sl], ysb[64:128, sl])

    nc.compile()
    return nc


def _prep_inputs(x, token_positions, Wq, Wk, Wv, Wo):
    # even/odd interleave permutation within each head (for rotate-half RoPE)
    perm = np.concatenate([np.arange(0, DK, 2), np.arange(1, DK, 2)])

    pos = np.asarray(token_positions).astype(np.float32)
    angles = THETA ** (-np.arange(32, dtype=np.float32) / 32.0)
    ang = pos[:, None] * angles[None, :]          # [S, 32]
    cos32 = np.cos(ang).T.astype(np.float32)      # [32, S]
    sin32 = np.sin(ang).T.astype(np.float32)
    cos128 = np.concatenate([cos32, cos32, cos32, cos32], axis=0)
    sin128 = np.concatenate([-sin32, sin32, -sin32, sin32], axis=0)
    cos128 = np.ascontiguousarray(cos128).astype(np.float16)
    sin128 = np.ascontiguousarray(sin128).astype(np.float16)

    identity = _round_fp32r(np.eye(128, dtype=np.float32))

    Wq = np.asarray(Wq, dtype=np.float32)
    Wk = np.asarray(Wk, dtype=np.float32)
    Wv = np.asarray(Wv, dtype=np.float32)
    Wo = np.asarray(Wo, dtype=np.float32)
    x = np.asarray(x, dtype=np.float32)

    in_maps = []
    for c in range(N_CORES):
        b = c // 4
        h0 = (c % 4) * HL
        esl = slice(h0 * DK, (h0 + HL) * DK)
        wq_h = Wq[esl].reshape(HL, DK, D)[:, perm].reshape(EL, D)
        wk_h = Wk[esl].reshape(HL, DK, D)[:, perm].reshape(EL, D)
        wv_h = Wv[esl]
        bf = lambda a: np.ascontiguousarray(a, dtype=np.float32).astype(np.float16)
        in_maps.append({
            "xT": bf(x[b].T),
            "wq": bf(wq_h.T),
            "wk": bf(wk_h.T),
            "wv": bf(wv_h.T),
            "wo": bf(Wo[:, esl].T),
            "cosT": cos128,
            "sinT": sin128,
            "ident": identity,
        })
    return in_maps


def kernel(x, token_positions, Wq, Wk, Wv, Wo, _trace=False):
    from concourse.bass_utils import run_bass_kernel_spmd

    global _compiled
    if _compiled is None:
        _compiled = _build()
    in_maps = _prep_inputs(x, token_positions, Wq, Wk, Wv, Wo)
    res = run_bass_kernel_spmd(_compiled, in_maps, list(range(N_CORES)),
                               trace=_trace)
    parts = [res.results[c]["y"].astype(np.float64) for c in range(N_CORES)]
    out = np.empty((2, S, D), dtype=np.float32)
    out[0] = (parts[0] + parts[1] + parts[2] + parts[3]).astype(np.float32)
    out[1] = (parts[4] + parts[5] + parts[6] + parts[7]).astype(np.float32)
    if _trace:
        return out, res
    return out
